# revision 5
# baseline (speedup 1.0000x reference)
"""Dense dot-product attention with key-length masking on 8 Trainium2 cores.

Problem: q,k,v [16, 2048, 128] fp32, valid_lens [16,1] int32.
  out = softmax(mask(q@k.T/sqrt(d))) @ v   (masked keys -> -1e6 before softmax)

Device algorithm (work unit = one chunk = (batch, 1024-query half, key-tile
range)):
- Host pre-transposes q,k to [d, seq] (fp16); the device never transposes.
- S^T tiles (keys on partitions): the key mask is a per-partition
  scale/bias folded into the exp() activation:
     E = exp(S_raw * scale_k + bias_k), scale_k = m_k/sqrt(d), bias_k = -30*(1-m_k)
  For valid_len==0 the host sets scale=bias=0 -> E=1 -> uniform softmax,
  matching the reference's where(mask, w, NEG) semantics exactly.
- O^T accumulates over the chunk's key tiles with V (fp16) stationary,
  E (fp16) moving.  Partial O^T (fp16) + partial softmax numerator sums
  (fp16 DVE pairwise tree) stream out per chunk; the host sums partials
  across chunks, finishes the 128-partition esum reduction, and divides.
- HAM warm-up: dummy bf16 matmuls run while the input DMAs stream, so
  the PE clock-gate is at 8/8 when real compute starts.

Work distribution (valid_lens-aware, single SPMD program):
- 32 units (16 batches x 2 query halves), work(unit) = ceil(L/128) key
  tiles (16 for L==0: uniform softmax must cover every key).  Units are
  CUT at arbitrary key-tile boundaries into chunks, so per-core work can
  be balanced to ceil(total/8) tiles: a greedy search picks slot
  capacities (e.g. [14,8,4,3,2,1]) and fills each core's slots with the
  largest remaining unit fragment.  Padded tiles (zero-filled k/v,
  scale=0/bias=-30 -> E~=0) cost engine time but are ~1% of the total.
- Slot order: a small slot opens (its input load gates compute start),
  the largest runs next (inputs stream during the opener), the smallest
  closes (short denominator tree + output tail).
"""

import math
import sys
import types

import numpy as np

import concourse.bass as bass
import concourse.mybir as mybir
import concourse.tile as tile
from concourse import bacc
from concourse.bass_utils import run_bass_kernel_spmd

B, Q, K, D = 16, 2048, 2048, 128
NCORES = 8
QCH = 1024         # queries per work unit
MM_N = 512         # moving-operand free dim per matmul
KT = K // 128      # max key tiles per unit
SCALE = 1.0 / math.sqrt(D)
NEG_BIAS = -30.0   # exp(-30) ~ 1e-13: invisible next to real softmax terms
WARMUP_MMS = 6     # dummy matmuls to lift the PE HAM clock-gate

F32 = mybir.dt.float32
F16 = mybir.dt.float16
BF16 = mybir.dt.bfloat16


def _install_hook_stub():
    """bass_utils' axon trace path imports antenv.axon_hooks, which is not
    shipped in this container.  Provide a no-op stub so an ambient
    BASS_TRACE=1 doesn't crash; test harnesses may overwrite the hook."""
    if "antenv.axon_hooks" in sys.modules:
        return
    mod = types.ModuleType("antenv.axon_hooks")
    _hook = [None]
    mod.set_axon_ntff_profile_hook = lambda h: _hook.__setitem__(0, h)
    mod.get_axon_ntff_profile_hook = lambda: _hook[0]
    sys.modules["antenv.axon_hooks"] = mod


_install_hook_stub()

_build_cache = {}
_sched_cache = {}
last_result = None  # BassKernelResults of the most recent run (for harnesses)


def _search_caps(units):
    """Find slot capacities (desc) + per-core chunk assignment minimizing
    per-core tiles C, then padding, then slot count.  units: list of
    (need, unit_id).  Chunks may split a unit at any key-tile boundary."""
    total = sum(n for n, _ in units)
    cmin = max((total + NCORES - 1) // NCORES, 1)

    def pack(caps):
        pool = [[n, u] for n, u in sorted(units, reverse=True)]
        pad = 0
        percore = [[] for _ in range(NCORES)]
        for cap in caps:
            for c in range(NCORES):
                pool.sort(key=lambda x: -x[0])
                if pool and pool[0][0] > 0:
                    take = min(cap, pool[0][0])
                    percore[c].append((pool[0][1], take))
                    pool[0][0] -= take
                    pad += cap - take
                else:
                    percore[c].append(None)
                    pad += cap
        if sum(p[0] for p in pool):
            return None
        return pad, percore

    def gen(remaining, parts, maxp):
        if parts == 1:
            if 1 <= remaining <= maxp:
                yield (remaining,)
            return
        for p in range(min(maxp, remaining - (parts - 1)), 0, -1):
            for rest in gen(remaining - p, parts - 1, p):
                yield (p,) + rest

    best = None
    for C in range(cmin, cmin + 4):
        for m in range(4, 9):
            for caps in gen(C, m, KT):
                r = pack(caps)
                if r is None:
                    continue
                pad, percore = r
                key = (C, pad, m)
                if best is None or key < best[0]:
                    best = (key, caps, percore)
        if best is not None:
            break
    assert best is not None, "cap search failed"
    return best[1], best[2]


def _schedule(need):
    """-> (caps_in_program_order, assign[core][slot] = (unit, off, l) | None)"""
    units = sorted([(int(need[u]), u) for u in range(len(need))], reverse=True)
    caps, percore = _search_caps(units)
    m = len(caps)
    # program order: small opener, then descending, smallest last
    order = [m - 2] + list(range(m - 2)) + [m - 1] if m >= 2 else list(range(m))
    # chunks of one unit are taken big-slot-first by the greedy; assign
    # consecutive key-tile offsets in that same order
    next_off = {}
    assign = [[None] * m for _ in range(NCORES)]
    for j in range(m):          # greedy filled caps in desc order
        for c in range(NCORES):
            item = percore[c][j]
            if item is None:
                continue
            u, l = item
            off = next_off.get(u, 0)
            next_off[u] = off + l
            assign[c][j] = (u, off, l)
    caps_prog = tuple(caps[j] for j in order)
    assign_prog = [[assign[c][j] for j in order] for c in range(NCORES)]
    return caps_prog, assign_prog


def _build(caps):
    """One SPMD program: slot j processes caps[j] key tiles of one chunk."""
    nc = bacc.Bacc(num_devices=NCORES)
    m = len(caps)

    qT, kT, v, scbi, oT, esum = [], [], [], [], [], []
    for s, t in enumerate(caps):
        qT.append(nc.declare_dram_parameter(f"qT{s}", [D, QCH], F16, isOutput=False))
        kT.append(nc.declare_dram_parameter(f"kT{s}", [D, t * 128], F16, isOutput=False))
        v.append(nc.declare_dram_parameter(f"v{s}", [t * 128, D], F16, isOutput=False))
        scbi.append(nc.declare_dram_parameter(f"scbi{s}", [128, 2 * t], F32, isOutput=False))
        oT.append(nc.declare_dram_parameter(f"oT{s}", [D, QCH], F16, isOutput=True))
        esum.append(nc.declare_dram_parameter(f"esum{s}", [128, QCH], F16, isOutput=True))

    with tile.TileContext(nc) as tc:
        with (
            tc.tile_pool(name="consts", bufs=1) as consts,
            tc.tile_pool(name="inputs", bufs=2) as inpool,
            tc.tile_pool(name="epool", bufs=max(caps) + 4) as epool,
            tc.tile_pool(name="treep", bufs=3) as treepool,
            tc.tile_pool(name="osb", bufs=2) as opool,
            tc.tile_pool(name="sps", bufs=3, space="PSUM") as pspool,
            tc.tile_pool(name="oacc", bufs=1, space="PSUM") as psacc,
        ):
            # --- HAM warm-up: dummy bf16 matmuls while input DMAs stream ---
            wsrc = consts.tile([128, MM_N], BF16)
            nc.gpsimd.memset(wsrc[:], 1.0)
            for w in range(WARMUP_MMS):
                if w % 2 == 0:
                    wps = pspool.tile([128, QCH], F32, tag="s")
                nc.tensor.matmul(
                    wps[:, (w % 2) * MM_N : (w % 2) * MM_N + MM_N],
                    wsrc[:, :128],
                    wsrc[:],
                    start=True,
                    stop=True,
                    skip_group_check=True,
                )

            for s in range(m):
                t = caps[s]
                qT_sb = inpool.tile([128, QCH], F16, tag="qT")
                kT_sb = inpool.tile([128, t * 128], F16, tag="kT")
                v_sb = inpool.tile([128, t, D], F16, tag="v")
                scbi_sb = inpool.tile([128, 2 * t], F32, tag="scbi")
                v_dram = v[s].rearrange("(i p) d -> p i d", p=128)
                pk = 5  # key tiles per DMA piece
                ntp = (t + pk - 1) // pk
                if s == 0:
                    # opener: tiny input set entirely on the two HWDGE rings
                    nc.sync.dma_start(out=scbi_sb[:], in_=scbi[s][:, :])
                    nc.sync.dma_start(out=qT_sb[:, :MM_N], in_=qT[s][:, :MM_N])
                    nc.scalar.dma_start(out=qT_sb[:, MM_N:], in_=qT[s][:, MM_N:])
                    nc.sync.dma_start(out=kT_sb[:], in_=kT[s][:, :])
                    nc.sync.dma_start(out=v_sb[:], in_=v_dram[:])
                else:
                    nc.sync.dma_start(out=scbi_sb[:], in_=scbi[s][:, :])
                    nc.sync.dma_start(out=qT_sb[:], in_=qT[s][:, :])
                    for j in range(ntp):
                        klo, khi = j * pk * 128, min(t * 128, (j + 1) * pk * 128)
                        eng = nc.sync if j == 0 else nc.gpsimd
                        eng.dma_start(out=kT_sb[:, klo:khi], in_=kT[s][:, klo:khi])
                        tlo, thi = j * pk, min(t, (j + 1) * pk)
                        eng2 = nc.gpsimd if j == 0 else nc.sync
                        eng2.dma_start(out=v_sb[:, tlo:thi, :], in_=v_dram[:, tlo:thi, :])

                etiles = []
                o_ps = psacc.tile([128, QCH], F32, tag="o")
                for i in range(t):
                    s_ps = pspool.tile([128, QCH], F32, tag="s")
                    for h in range(QCH // MM_N):
                        nc.tensor.matmul(
                            s_ps[:, bass.ts(h, MM_N)],
                            kT_sb[:, bass.ts(i, 128)],
                            qT_sb[:, bass.ts(h, MM_N)],
                            start=True,
                            stop=True,
                        )
                    e_sb = epool.tile([128, QCH], F16, tag="e")
                    etiles.append(e_sb)
                    # slot0/tile0: exp in halves so the pipeline ignites on
                    # the first qT piece instead of waiting for both
                    parts = [bass.ts(p, MM_N) for p in range(2)] if (s == 0 and i == 0) else [slice(None)]
                    for pr in parts:
                        nc.scalar.activation(
                            e_sb[:, pr],
                            s_ps[:, pr],
                            mybir.ActivationFunctionType.Exp,
                            bias=scbi_sb[:, t + i : t + i + 1],
                            scale=scbi_sb[:, i : i + 1],
                        )
                    for h in range(QCH // MM_N):
                        nc.tensor.matmul(
                            o_ps[:, bass.ts(h, MM_N)],
                            v_sb[:, i, :],
                            e_sb[:, bass.ts(h, MM_N)],
                            start=(i == 0),
                            stop=(i == t - 1),
                        )

                # denominator: DVE pairwise fp16 tree (2x mode) down to one
                # [128, QCH] survivor; host finishes the partition sum
                cur = [e[:] for e in etiles]
                if len(cur) > 1:
                    tr = treepool.tile([128, (t + 1) // 2, QCH], F16, tag="tr")
                    nxt = []
                    for j in range(len(cur) // 2):
                        # first level alternates DVE / Pool (both read SBUF)
                        eng = nc.vector if j % 2 == 0 else nc.gpsimd
                        eng.tensor_add(tr[:, j, :], cur[2 * j], cur[2 * j + 1])
                        nxt.append(tr[:, j, :])
                    if len(cur) % 2:
                        nxt.append(cur[-1])
                    cur = nxt
                    while len(cur) > 1:
                        nxt = []
                        for j in range(len(cur) // 2):
                            nc.vector.tensor_add(cur[2 * j], cur[2 * j], cur[2 * j + 1])
                            nxt.append(cur[2 * j])
                        if len(cur) % 2:
                            nxt.append(cur[-1])
                        cur = nxt
                esum_eng = nc.sync if s == m - 1 else nc.gpsimd
                esum_eng.dma_start(out=esum[s][:, :], in_=cur[0])

                # O^T: PSUM f32 -> SBUF f16 (DVE: Pool has no PSUM port), in
                # halves so the first DMA overlaps the second copy
                o_sb = opool.tile([128, QCH], F16, tag="osb")
                for h in range(2):
                    hs = bass.ts(h, QCH // 2)
                    nc.vector.tensor_copy(o_sb[:, hs], o_ps[:, hs])
                    nc.sync.dma_start(out=oT[s][:, hs], in_=o_sb[:, hs])

    nc.compile()
    return nc


def kernel(q, k, v, valid_lens):
    q = np.ascontiguousarray(q, dtype=np.float32)
    k = np.ascontiguousarray(k, dtype=np.float32)
    v = np.ascontiguousarray(v, dtype=np.float32)
    L = np.asarray(valid_lens).reshape(-1).astype(np.int64)

    # per-unit key-tile need; L==0 must cover all keys (uniform softmax)
    need_b = np.where(L == 0, KT, np.minimum(KT, (L + 127) // 128)).astype(np.int64)
    # units: (batch, q-half) -> unit id u = 2*b + h
    need = np.repeat(need_b, Q // QCH)

    skey = tuple(need.tolist())
    if skey not in _sched_cache:
        _sched_cache[skey] = _schedule(need)
    caps, assign = _sched_cache[skey]

    if caps not in _build_cache:
        _build_cache[caps] = _build(caps)
    nc = _build_cache[caps]

    qh = q.astype(np.float16)
    kh = k.astype(np.float16)
    vh = v.astype(np.float16)
    kT_full = np.ascontiguousarray(kh.transpose(0, 2, 1))  # [B, D, K]
    qT_full = np.ascontiguousarray(
        qh.reshape(B, Q // QCH, QCH, D).transpose(0, 1, 3, 2)
    )  # [B, halves, D, QCH]

    kidx = np.arange(K)
    in_maps = []
    for c in range(NCORES):
        im = {}
        for s, t in enumerate(caps):
            chunk = assign[c][s]
            sc = np.zeros((128, t), np.float32)
            bi = np.full((128, t), np.float32(NEG_BIAS))
            if chunk is None:
                im[f"qT{s}"] = np.zeros((D, QCH), np.float16)
                im[f"kT{s}"] = np.zeros((D, t * 128), np.float16)
                im[f"v{s}"] = np.zeros((t * 128, D), np.float16)
            else:
                u, off, l = chunk
                b, h = u // 2, u % 2
                lb = int(L[b])
                im[f"qT{s}"] = qT_full[b, h]
                kT_c = np.zeros((D, t * 128), np.float16)
                kT_c[:, : l * 128] = kT_full[b][:, off * 128 : (off + l) * 128]
                im[f"kT{s}"] = kT_c
                v_c = np.zeros((t * 128, D), np.float16)
                v_c[: l * 128] = vh[b][off * 128 : (off + l) * 128]
                im[f"v{s}"] = v_c
                keys = (off * 128 + kidx[: l * 128]).reshape(l, 128).T  # [128, l]
                if lb == 0:
                    sc[:, :l] = 0.0
                    bi[:, :l] = 0.0
                else:
                    mk = (keys < lb).astype(np.float32)
                    sc[:, :l] = mk * np.float32(SCALE)
                    bi[:, :l] = (1.0 - mk) * np.float32(NEG_BIAS)
            im[f"scbi{s}"] = np.ascontiguousarray(
                np.concatenate([sc, bi], axis=1)
            )
        in_maps.append(im)

    res = run_bass_kernel_spmd(nc, in_maps, list(range(NCORES)))
    global last_result
    last_result = res

    acc_o = np.zeros((B * 2, D, QCH), np.float32)
    acc_d = np.zeros((B * 2, QCH), np.float32)
    for c in range(NCORES):
        r = res.results[c]
        for s in range(len(caps)):
            chunk = assign[c][s]
            if chunk is None:
                continue
            u = chunk[0]
            acc_o[u] += r[f"oT{s}"].astype(np.float32)
            acc_d[u] += r[f"esum{s}"].astype(np.float32).sum(axis=0)

    out = np.empty((B, Q, D), np.float32)
    for u in range(B * 2):
        b, h = u // 2, u % 2
        out[b, h * QCH : (h + 1) * QCH] = (acc_o[u] / acc_d[u][None, :]).T
    return out


# revision 7
# speedup vs baseline: 1.0353x; 1.0353x over previous
"""Dense dot-product attention with key-length masking on 8 Trainium2 cores.

Problem: q,k,v [16, 2048, 128] fp32, valid_lens [16,1] int32.
  out = softmax(mask(q@k.T/sqrt(d))) @ v   (masked keys -> -1e6 before softmax)

Device algorithm (work unit = one chunk = (batch, 1024-query half, key-tile
range)):
- Host pre-transposes q,k to [d, seq] (fp16); the device never transposes.
- S^T tiles (keys on partitions): the key mask is a per-partition
  scale/bias folded into the exp() activation:
     E = exp(S_raw * scale_k + bias_k), scale_k = m_k/sqrt(d), bias_k = -30*(1-m_k)
  For valid_len==0 the host sets scale=bias=0 -> E=1 -> uniform softmax,
  matching the reference's where(mask, w, NEG) semantics exactly.
- O^T accumulates over the chunk's key tiles with V (fp16) stationary,
  E (fp16) moving.  Partial O^T (fp16) + partial softmax numerator sums
  (fp16 DVE pairwise tree) stream out per chunk; the host sums partials
  across chunks, finishes the 128-partition esum reduction, and divides.
- HAM warm-up: dummy bf16 matmuls run while the input DMAs stream, so
  the PE clock-gate is at 8/8 when real compute starts.

Work distribution (valid_lens-aware, single SPMD program):
- 32 units (16 batches x 2 query halves), work(unit) = ceil(L/128) key
  tiles (16 for L==0: uniform softmax must cover every key).  Units are
  CUT at arbitrary key-tile boundaries into chunks, so per-core work can
  be balanced to ceil(total/8) tiles: a greedy search picks slot
  capacities (e.g. [14,8,4,3,2,1]) and fills each core's slots with the
  largest remaining unit fragment.  Padded tiles (zero-filled k/v,
  scale=0/bias=-30 -> E~=0) cost engine time but are ~1% of the total.
- Slot order: a small slot opens (its input load gates compute start),
  the largest runs next (inputs stream during the opener), the smallest
  closes (short denominator tree + output tail).
"""

import math
import sys
import types

import numpy as np

import concourse.bass as bass
import concourse.mybir as mybir
import concourse.tile as tile
from concourse import bacc
from concourse.bass_utils import run_bass_kernel_spmd

B, Q, K, D = 16, 2048, 2048, 128
NCORES = 8
QCH = 1024         # queries per work unit
MM_N = 512         # moving-operand free dim per matmul
KT = K // 128      # max key tiles per unit
SCALE = 1.0 / math.sqrt(D)
NEG_BIAS = -30.0   # exp(-30) ~ 1e-13: invisible next to real softmax terms
WARMUP_MMS = 6     # dummy matmuls to lift the PE HAM clock-gate

F32 = mybir.dt.float32
F16 = mybir.dt.float16
BF16 = mybir.dt.bfloat16


def _install_hook_stub():
    """bass_utils' axon trace path imports antenv.axon_hooks, which is not
    shipped in this container.  Provide a no-op stub so an ambient
    BASS_TRACE=1 doesn't crash; test harnesses may overwrite the hook."""
    if "antenv.axon_hooks" in sys.modules:
        return
    mod = types.ModuleType("antenv.axon_hooks")
    _hook = [None]
    mod.set_axon_ntff_profile_hook = lambda h: _hook.__setitem__(0, h)
    mod.get_axon_ntff_profile_hook = lambda: _hook[0]
    sys.modules["antenv.axon_hooks"] = mod


_install_hook_stub()

_build_cache = {}
_sched_cache = {}
last_result = None  # BassKernelResults of the most recent run (for harnesses)


def _search_caps(units):
    """Find slot capacities (desc) + per-core chunk assignment minimizing
    per-core tiles C, then padding, then slot count.  units: list of
    (need, unit_id).  Chunks may split a unit at any key-tile boundary."""
    total = sum(n for n, _ in units)
    cmin = max((total + NCORES - 1) // NCORES, 1)

    def pack(caps):
        pool = [[n, u] for n, u in sorted(units, reverse=True)]
        pad = 0
        percore = [[] for _ in range(NCORES)]
        for cap in caps:
            for c in range(NCORES):
                pool.sort(key=lambda x: -x[0])
                if pool and pool[0][0] > 0:
                    take = min(cap, pool[0][0])
                    percore[c].append((pool[0][1], take))
                    pool[0][0] -= take
                    pad += cap - take
                else:
                    percore[c].append(None)
                    pad += cap
        if sum(p[0] for p in pool):
            return None
        return pad, percore

    def gen(remaining, parts, maxp):
        if parts == 1:
            if 1 <= remaining <= maxp:
                yield (remaining,)
            return
        for p in range(min(maxp, remaining - (parts - 1)), 0, -1):
            for rest in gen(remaining - p, parts - 1, p):
                yield (p,) + rest

    best = None
    for C in range(cmin, cmin + 4):
        for m in range(4, 9):
            for caps in gen(C, m, KT):
                r = pack(caps)
                if r is None:
                    continue
                pad, percore = r
                key = (C, pad, m)
                if best is None or key < best[0]:
                    best = (key, caps, percore)
        if best is not None:
            break
    assert best is not None, "cap search failed"
    return best[1], best[2]


def _schedule(need):
    """-> (caps_in_program_order, assign[core][slot] = (unit, off, l) | None)"""
    units = sorted([(int(need[u]), u) for u in range(len(need))], reverse=True)
    caps, percore = _search_caps(units)
    m = len(caps)
    # program order: a mid-size opener (compute covers the biggest slot's
    # input load), then descending, smallest last
    if m >= 2:
        opener = min(range(m - 1), key=lambda j: (abs(caps[j] - 4), j))
        order = [opener] + [j for j in range(m - 1) if j != opener] + [m - 1]
    else:
        order = list(range(m))
    # chunks of one unit are taken big-slot-first by the greedy; assign
    # consecutive key-tile offsets in that same order
    next_off = {}
    assign = [[None] * m for _ in range(NCORES)]
    for j in range(m):          # greedy filled caps in desc order
        for c in range(NCORES):
            item = percore[c][j]
            if item is None:
                continue
            u, l = item
            off = next_off.get(u, 0)
            next_off[u] = off + l
            assign[c][j] = (u, off, l)
    caps_prog = tuple(caps[j] for j in order)
    assign_prog = [[assign[c][j] for j in order] for c in range(NCORES)]
    return caps_prog, assign_prog


def _build(caps):
    """One SPMD program: slot j processes caps[j] key tiles of one chunk."""
    nc = bacc.Bacc(num_devices=NCORES)
    m = len(caps)

    scbi_all = nc.declare_dram_parameter("scbi", [128, 2 * sum(caps)], F32, isOutput=False)
    NQ = QCH // 128
    inp, oT, esum = [], [], []
    for s, t in enumerate(caps):
        # packed input: NQ groups of qT cols, t of kT, t of v (all [128,128])
        inp.append(nc.declare_dram_parameter(f"in{s}", [128, NQ + 2 * t, 128], F16, isOutput=False))
        oT.append(nc.declare_dram_parameter(f"oT{s}", [D, QCH], F16, isOutput=True))
        esum.append(nc.declare_dram_parameter(f"esum{s}", [128, QCH], F16, isOutput=True))

    with tile.TileContext(nc) as tc:
        with (
            tc.tile_pool(name="consts", bufs=1) as consts,
            tc.tile_pool(name="inputs", bufs=4) as inpool,
            tc.tile_pool(name="epool", bufs=max(caps) + 4) as epool,
            tc.tile_pool(name="treep", bufs=3) as treepool,
            tc.tile_pool(name="osb", bufs=2) as opool,
            tc.tile_pool(name="sps", bufs=3, space="PSUM") as pspool,
            tc.tile_pool(name="oacc", bufs=1, space="PSUM") as psacc,
        ):
            # --- HAM warm-up: dummy bf16 matmuls while input DMAs stream ---
            wsrc = consts.tile([128, MM_N], BF16)
            nc.gpsimd.memset(wsrc[:], 1.0)
            scbi_sb = consts.tile([128, 2 * sum(caps)], F32)
            nc.sync.dma_start(out=scbi_sb[:], in_=scbi_all[:, :])
            scbi_off = [2 * sum(caps[:j]) for j in range(m)]
            for w in range(WARMUP_MMS):
                if w % 2 == 0:
                    wps = pspool.tile([128, QCH], F32, tag="s")
                nc.tensor.matmul(
                    wps[:, (w % 2) * MM_N : (w % 2) * MM_N + MM_N],
                    wsrc[:, :128],
                    wsrc[:],
                    start=True,
                    stop=True,
                    skip_group_check=True,
                )

            for s in range(m):
                t = caps[s]
                in_sb = inpool.tile([128, NQ + 2 * t, 128], F16, tag="in")
                qT_sb = in_sb.reshape([128, (NQ + 2 * t) * 128])
                so = scbi_off[s]
                # one packed input DMA per slot (few, large transfers: the
                # HWDGE ring serializes per-DMA latency otherwise)
                nc.sync.dma_start(out=in_sb[:, :, :], in_=inp[s][:, :, :])

                etiles = []
                o_ps = psacc.tile([128, QCH], F32, tag="o")
                for i in range(t):
                    s_ps = pspool.tile([128, QCH], F32, tag="s")
                    for h in range(QCH // MM_N):
                        nc.tensor.matmul(
                            s_ps[:, bass.ts(h, MM_N)],
                            qT_sb[:, (NQ + i) * 128 : (NQ + i + 1) * 128],
                            qT_sb[:, bass.ts(h, MM_N)],
                            start=True,
                            stop=True,
                        )
                    e_sb = epool.tile([128, QCH], F16, tag="e")
                    etiles.append(e_sb)
                    # slot0/tile0: exp in halves so the pipeline ignites on
                    # the first qT piece instead of waiting for both
                    parts = [bass.ts(p, MM_N) for p in range(2)] if (s == 0 and i == 0) else [slice(None)]
                    for pr in parts:
                        nc.scalar.activation(
                            e_sb[:, pr],
                            s_ps[:, pr],
                            mybir.ActivationFunctionType.Exp,
                            bias=scbi_sb[:, so + t + i : so + t + i + 1],
                            scale=scbi_sb[:, so + i : so + i + 1],
                        )
                    for h in range(QCH // MM_N):
                        nc.tensor.matmul(
                            o_ps[:, bass.ts(h, MM_N)],
                            qT_sb[:, (NQ + t + i) * 128 : (NQ + t + i + 1) * 128],
                            e_sb[:, bass.ts(h, MM_N)],
                            start=(i == 0),
                            stop=(i == t - 1),
                        )

                # denominator: DVE pairwise fp16 tree (2x mode) down to one
                # [128, QCH] survivor; host finishes the partition sum
                cur = [e[:] for e in etiles]
                if len(cur) > 1:
                    tr = treepool.tile([128, (t + 1) // 2, QCH], F16, tag="tr")
                    nxt = []
                    for j in range(len(cur) // 2):
                        nc.vector.tensor_add(tr[:, j, :], cur[2 * j], cur[2 * j + 1])
                        nxt.append(tr[:, j, :])
                    if len(cur) % 2:
                        nxt.append(cur[-1])
                    cur = nxt
                    while len(cur) > 1:
                        nxt = []
                        for j in range(len(cur) // 2):
                            nc.vector.tensor_add(cur[2 * j], cur[2 * j], cur[2 * j + 1])
                            nxt.append(cur[2 * j])
                        if len(cur) % 2:
                            nxt.append(cur[-1])
                        cur = nxt
                esum_eng = nc.sync if s == m - 1 else nc.gpsimd
                esum_eng.dma_start(out=esum[s][:, :], in_=cur[0])

                # O^T: PSUM f32 -> SBUF f16 (DVE: Pool has no PSUM port), in
                # halves so the first DMA overlaps the second copy
                o_sb = opool.tile([128, QCH], F16, tag="osb")
                for h in range(2):
                    hs = bass.ts(h, QCH // 2)
                    nc.vector.tensor_copy(o_sb[:, hs], o_ps[:, hs])
                nc.sync.dma_start(out=oT[s][:, :], in_=o_sb[:])

    nc.compile()
    return nc


def kernel(q, k, v, valid_lens):
    q = np.ascontiguousarray(q, dtype=np.float32)
    k = np.ascontiguousarray(k, dtype=np.float32)
    v = np.ascontiguousarray(v, dtype=np.float32)
    L = np.asarray(valid_lens).reshape(-1).astype(np.int64)

    # per-unit key-tile need; L==0 must cover all keys (uniform softmax)
    need_b = np.where(L == 0, KT, np.minimum(KT, (L + 127) // 128)).astype(np.int64)
    # units: (batch, q-half) -> unit id u = 2*b + h
    need = np.repeat(need_b, Q // QCH)

    skey = tuple(need.tolist())
    if skey not in _sched_cache:
        _sched_cache[skey] = _schedule(need)
    caps, assign = _sched_cache[skey]

    if caps not in _build_cache:
        _build_cache[caps] = _build(caps)
    nc = _build_cache[caps]

    qh = q.astype(np.float16)
    kh = k.astype(np.float16)
    vh = v.astype(np.float16)
    kT_full = np.ascontiguousarray(kh.transpose(0, 2, 1))  # [B, D, K]
    qT_full = np.ascontiguousarray(
        qh.reshape(B, Q // QCH, QCH, D).transpose(0, 1, 3, 2)
    )  # [B, halves, D, QCH]

    kidx = np.arange(K)
    NQ = QCH // 128
    in_maps = []
    for c in range(NCORES):
        im = {}
        scbi_parts = []
        for s, t in enumerate(caps):
            chunk = assign[c][s]
            sc = np.zeros((128, t), np.float32)
            bi = np.full((128, t), np.float32(NEG_BIAS))
            pk_in = np.zeros((128, NQ + 2 * t, 128), np.float16)
            if chunk is not None:
                u, off, l = chunk
                b, h = u // 2, u % 2
                lb = int(L[b])
                pk_in[:, :NQ, :] = qT_full[b, h].reshape(D, NQ, 128).transpose(0, 1, 2)
                kslc = kT_full[b][:, off * 128 : (off + l) * 128]  # [D, l*128]
                pk_in[:, NQ : NQ + l, :] = kslc.reshape(D, l, 128)
                vslc = vh[b][off * 128 : (off + l) * 128]  # [l*128, D]
                # v packed partition-major: [p, i, d] = v[i*128+p, d]
                pk_in[:, NQ + t : NQ + t + l, :] = vslc.reshape(l, 128, D).transpose(1, 0, 2)
                keys = (off * 128 + kidx[: l * 128]).reshape(l, 128).T  # [128, l]
                if lb == 0:
                    sc[:, :l] = 0.0
                    bi[:, :l] = 0.0
                else:
                    mk = (keys < lb).astype(np.float32)
                    sc[:, :l] = mk * np.float32(SCALE)
                    bi[:, :l] = (1.0 - mk) * np.float32(NEG_BIAS)
            im[f"in{s}"] = pk_in
            scbi_parts.append(np.concatenate([sc, bi], axis=1))
        im["scbi"] = np.ascontiguousarray(np.concatenate(scbi_parts, axis=1))
        in_maps.append(im)

    res = run_bass_kernel_spmd(nc, in_maps, list(range(NCORES)))
    global last_result
    last_result = res

    acc_o = np.zeros((B * 2, D, QCH), np.float32)
    acc_d = np.zeros((B * 2, QCH), np.float32)
    for c in range(NCORES):
        r = res.results[c]
        for s in range(len(caps)):
            chunk = assign[c][s]
            if chunk is None:
                continue
            u = chunk[0]
            acc_o[u] += r[f"oT{s}"].astype(np.float32)
            acc_d[u] += r[f"esum{s}"].astype(np.float32).sum(axis=0)

    out = np.empty((B, Q, D), np.float32)
    for u in range(B * 2):
        b, h = u // 2, u % 2
        out[b, h * QCH : (h + 1) * QCH] = (acc_o[u] / acc_d[u][None, :]).T
    return out


# revision 8
# speedup vs baseline: 1.0515x; 1.0156x over previous
"""Dense dot-product attention with key-length masking on 8 Trainium2 cores.

Problem: q,k,v [16, 2048, 128] fp32, valid_lens [16,1] int32.
  out = softmax(mask(q@k.T/sqrt(d))) @ v   (masked keys -> -1e6 before softmax)

Device algorithm (work unit = one chunk = (batch, 1024-query half, key-tile
range)):
- Host pre-transposes q,k to [d, seq] (fp16); the device never transposes.
- S^T tiles (keys on partitions): the key mask is a per-partition
  scale/bias folded into the exp() activation:
     E = exp(S_raw * scale_k + bias_k), scale_k = m_k/sqrt(d), bias_k = -30*(1-m_k)
  For valid_len==0 the host sets scale=bias=0 -> E=1 -> uniform softmax,
  matching the reference's where(mask, w, NEG) semantics exactly.
- O^T accumulates over the chunk's key tiles with V (fp16) stationary,
  E (fp16) moving.  Partial O^T (fp16) + partial softmax numerator sums
  (fp16 DVE pairwise tree) stream out per chunk; the host sums partials
  across chunks, finishes the 128-partition esum reduction, and divides.
- HAM warm-up: dummy bf16 matmuls run while the input DMAs stream, so
  the PE clock-gate is at 8/8 when real compute starts.

Work distribution (valid_lens-aware, single SPMD program):
- 32 units (16 batches x 2 query halves), work(unit) = ceil(L/128) key
  tiles (16 for L==0: uniform softmax must cover every key).  Units are
  CUT at arbitrary key-tile boundaries into chunks, so per-core work can
  be balanced to ceil(total/8) tiles: a greedy search picks slot
  capacities (e.g. [14,8,4,3,2,1]) and fills each core's slots with the
  largest remaining unit fragment.  Padded tiles (zero-filled k/v,
  scale=0/bias=-30 -> E~=0) cost engine time but are ~1% of the total.
- Slot order: a small slot opens (its input load gates compute start),
  the largest runs next (inputs stream during the opener), the smallest
  closes (short denominator tree + output tail).
"""

import math
import sys
import types

import numpy as np

import concourse.bass as bass
import concourse.mybir as mybir
import concourse.tile as tile
from concourse import bacc
from concourse.bass_utils import run_bass_kernel_spmd

B, Q, K, D = 16, 2048, 2048, 128
NCORES = 8
QCH = 1024         # queries per work unit
MM_N = 512         # moving-operand free dim per matmul
KT = K // 128      # max key tiles per unit
SCALE = 1.0 / math.sqrt(D)
NEG_BIAS = -30.0   # exp(-30) ~ 1e-13: invisible next to real softmax terms
WARMUP_MMS = 6     # dummy matmuls to lift the PE HAM clock-gate

F32 = mybir.dt.float32
F16 = mybir.dt.float16
BF16 = mybir.dt.bfloat16


def _install_hook_stub():
    """bass_utils' axon trace path imports antenv.axon_hooks, which is not
    shipped in this container.  Provide a no-op stub so an ambient
    BASS_TRACE=1 doesn't crash; test harnesses may overwrite the hook."""
    if "antenv.axon_hooks" in sys.modules:
        return
    mod = types.ModuleType("antenv.axon_hooks")
    _hook = [None]
    mod.set_axon_ntff_profile_hook = lambda h: _hook.__setitem__(0, h)
    mod.get_axon_ntff_profile_hook = lambda: _hook[0]
    sys.modules["antenv.axon_hooks"] = mod


_install_hook_stub()

_build_cache = {}
_sched_cache = {}
last_result = None  # BassKernelResults of the most recent run (for harnesses)


def _search_caps(units):
    """Find slot capacities (desc) + per-core chunk assignment minimizing
    per-core tiles C, then padding, then slot count.  units: list of
    (need, unit_id).  Chunks may split a unit at any key-tile boundary."""
    total = sum(n for n, _ in units)
    cmin = max((total + NCORES - 1) // NCORES, 1)

    def pack(caps):
        pool = [[n, u] for n, u in sorted(units, reverse=True)]
        pad = 0
        percore = [[] for _ in range(NCORES)]
        for cap in caps:
            for c in range(NCORES):
                pool.sort(key=lambda x: -x[0])
                if pool and pool[0][0] > 0:
                    take = min(cap, pool[0][0])
                    percore[c].append((pool[0][1], take))
                    pool[0][0] -= take
                    pad += cap - take
                else:
                    percore[c].append(None)
                    pad += cap
        if sum(p[0] for p in pool):
            return None
        return pad, percore

    def gen(remaining, parts, maxp):
        if parts == 1:
            if 1 <= remaining <= maxp:
                yield (remaining,)
            return
        for p in range(min(maxp, remaining - (parts - 1)), 0, -1):
            for rest in gen(remaining - p, parts - 1, p):
                yield (p,) + rest

    best = None
    for C in range(cmin, cmin + 4):
        for m in range(4, 9):
            for caps in gen(C, m, KT):
                r = pack(caps)
                if r is None:
                    continue
                pad, percore = r
                key = (C, pad, m)
                if best is None or key < best[0]:
                    best = (key, caps, percore)
        if best is not None:
            break
    assert best is not None, "cap search failed"
    return best[1], best[2]


def _schedule(need):
    """-> (caps_in_program_order, assign[core][slot] = (unit, off, l) | None)"""
    units = sorted([(int(need[u]), u) for u in range(len(need))], reverse=True)
    caps, percore = _search_caps(units)
    m = len(caps)
    # program order: a mid-size opener (compute covers the biggest slot's
    # input load), then descending, smallest last
    if m >= 2:
        opener = min(range(m - 1), key=lambda j: (abs(caps[j] - 4), j))
        order = [opener] + [j for j in range(m - 1) if j != opener] + [m - 1]
    else:
        order = list(range(m))
    # chunks of one unit are taken big-slot-first by the greedy; assign
    # consecutive key-tile offsets in that same order
    next_off = {}
    assign = [[None] * m for _ in range(NCORES)]
    for j in range(m):          # greedy filled caps in desc order
        for c in range(NCORES):
            item = percore[c][j]
            if item is None:
                continue
            u, l = item
            off = next_off.get(u, 0)
            next_off[u] = off + l
            assign[c][j] = (u, off, l)
    caps_prog = tuple(caps[j] for j in order)
    assign_prog = [[assign[c][j] for j in order] for c in range(NCORES)]
    return caps_prog, assign_prog


def _build(caps):
    """One SPMD program: slot j processes caps[j] key tiles of one chunk."""
    nc = bacc.Bacc(num_devices=NCORES)
    m = len(caps)

    scbi_all = nc.declare_dram_parameter("scbi", [128, 2 * sum(caps)], F32, isOutput=False)
    NQ = QCH // 128
    inp, oT, esum = [], [], []
    for s, t in enumerate(caps):
        # packed input: NQ groups of qT cols, t of kT, t of v (all [128,128])
        inp.append(nc.declare_dram_parameter(f"in{s}", [128, (NQ + 2 * t) * 128], F16, isOutput=False))
        oT.append(nc.declare_dram_parameter(f"oT{s}", [D, QCH], F16, isOutput=True))
        esum.append(nc.declare_dram_parameter(f"esum{s}", [128, QCH], F16, isOutput=True))

    with tile.TileContext(nc) as tc:
        with (
            tc.tile_pool(name="consts", bufs=1) as consts,
            tc.tile_pool(name="inputs", bufs=4) as inpool,
            tc.tile_pool(name="epool", bufs=max(caps) + 4) as epool,
            tc.tile_pool(name="treep", bufs=3) as treepool,
            tc.tile_pool(name="osb", bufs=2) as opool,
            tc.tile_pool(name="sps", bufs=3, space="PSUM") as pspool,
            tc.tile_pool(name="oacc", bufs=1, space="PSUM") as psacc,
        ):
            # --- HAM warm-up: dummy bf16 matmuls while input DMAs stream ---
            wsrc = consts.tile([128, MM_N], BF16)
            nc.gpsimd.memset(wsrc[:], 1.0)
            scbi_sb = consts.tile([128, 2 * sum(caps)], F32)
            nc.sync.dma_start(out=scbi_sb[:], in_=scbi_all[:, :])
            scbi_off = [2 * sum(caps[:j]) for j in range(m)]
            for w in range(WARMUP_MMS):
                if w % 2 == 0:
                    wps = pspool.tile([128, QCH], F32, tag="s")
                nc.tensor.matmul(
                    wps[:, (w % 2) * MM_N : (w % 2) * MM_N + MM_N],
                    wsrc[:, :128],
                    wsrc[:],
                    start=True,
                    stop=True,
                    skip_group_check=True,
                )

            for s in range(m):
                t = caps[s]
                qT_sb = inpool.tile([128, (NQ + 2 * t) * 128], F16, tag="in")
                so = scbi_off[s]
                # one packed input DMA per slot (few, large transfers: the
                # HWDGE ring serializes per-DMA latency otherwise)
                nc.sync.dma_start(out=qT_sb[:], in_=inp[s][:, :])

                etiles = []
                o_ps = psacc.tile([128, QCH], F32, tag="o")
                for i in range(t):
                    s_ps = pspool.tile([128, QCH], F32, tag="s")
                    for h in range(QCH // MM_N):
                        nc.tensor.matmul(
                            s_ps[:, bass.ts(h, MM_N)],
                            qT_sb[:, (NQ + i) * 128 : (NQ + i + 1) * 128],
                            qT_sb[:, bass.ts(h, MM_N)],
                            start=True,
                            stop=True,
                        )
                    e_sb = epool.tile([128, QCH], F16, tag="e")
                    etiles.append(e_sb)
                    # slot0/tile0: exp in halves so the pipeline ignites on
                    # the first qT piece instead of waiting for both
                    parts = [bass.ts(p, MM_N) for p in range(2)] if (s == 0 and i == 0) else [slice(None)]
                    for pr in parts:
                        nc.scalar.activation(
                            e_sb[:, pr],
                            s_ps[:, pr],
                            mybir.ActivationFunctionType.Exp,
                            bias=scbi_sb[:, so + t + i : so + t + i + 1],
                            scale=scbi_sb[:, so + i : so + i + 1],
                        )
                    for h in range(QCH // MM_N):
                        nc.tensor.matmul(
                            o_ps[:, bass.ts(h, MM_N)],
                            qT_sb[:, (NQ + t + i) * 128 : (NQ + t + i + 1) * 128],
                            e_sb[:, bass.ts(h, MM_N)],
                            start=(i == 0),
                            stop=(i == t - 1),
                        )

                # denominator: DVE pairwise fp16 tree (2x mode) down to one
                # [128, QCH] survivor; host finishes the partition sum
                cur = [e[:] for e in etiles]
                if len(cur) > 1:
                    tr = treepool.tile([128, (t + 1) // 2, QCH], F16, tag="tr")
                    nxt = []
                    for j in range(len(cur) // 2):
                        nc.vector.tensor_add(tr[:, j, :], cur[2 * j], cur[2 * j + 1])
                        nxt.append(tr[:, j, :])
                    if len(cur) % 2:
                        nxt.append(cur[-1])
                    cur = nxt
                    while len(cur) > 1:
                        nxt = []
                        for j in range(len(cur) // 2):
                            nc.vector.tensor_add(cur[2 * j], cur[2 * j], cur[2 * j + 1])
                            nxt.append(cur[2 * j])
                        if len(cur) % 2:
                            nxt.append(cur[-1])
                        cur = nxt
                esum_eng = nc.sync if s == m - 1 else nc.gpsimd
                esum_eng.dma_start(out=esum[s][:, :], in_=cur[0])

                # O^T: PSUM f32 -> SBUF f16 (DVE: Pool has no PSUM port), in
                # halves so the first DMA overlaps the second copy
                o_sb = opool.tile([128, QCH], F16, tag="osb")
                for h in range(2):
                    hs = bass.ts(h, QCH // 2)
                    nc.vector.tensor_copy(o_sb[:, hs], o_ps[:, hs])
                nc.sync.dma_start(out=oT[s][:, :], in_=o_sb[:])

    nc.compile()
    return nc


def kernel(q, k, v, valid_lens):
    q = np.ascontiguousarray(q, dtype=np.float32)
    k = np.ascontiguousarray(k, dtype=np.float32)
    v = np.ascontiguousarray(v, dtype=np.float32)
    L = np.asarray(valid_lens).reshape(-1).astype(np.int64)

    # per-unit key-tile need; L==0 must cover all keys (uniform softmax)
    need_b = np.where(L == 0, KT, np.minimum(KT, (L + 127) // 128)).astype(np.int64)
    # units: (batch, q-half) -> unit id u = 2*b + h
    need = np.repeat(need_b, Q // QCH)

    skey = tuple(need.tolist())
    if skey not in _sched_cache:
        _sched_cache[skey] = _schedule(need)
    caps, assign = _sched_cache[skey]

    if caps not in _build_cache:
        _build_cache[caps] = _build(caps)
    nc = _build_cache[caps]

    qh = q.astype(np.float16)
    kh = k.astype(np.float16)
    vh = v.astype(np.float16)
    kT_full = np.ascontiguousarray(kh.transpose(0, 2, 1))  # [B, D, K]
    qT_full = np.ascontiguousarray(
        qh.reshape(B, Q // QCH, QCH, D).transpose(0, 1, 3, 2)
    )  # [B, halves, D, QCH]

    kidx = np.arange(K)
    NQ = QCH // 128
    in_maps = []
    for c in range(NCORES):
        im = {}
        scbi_parts = []
        for s, t in enumerate(caps):
            chunk = assign[c][s]
            sc = np.zeros((128, t), np.float32)
            bi = np.full((128, t), np.float32(NEG_BIAS))
            pk_in = np.zeros((128, NQ + 2 * t, 128), np.float16)
            if chunk is not None:
                u, off, l = chunk
                b, h = u // 2, u % 2
                lb = int(L[b])
                pk_in[:, :NQ, :] = qT_full[b, h].reshape(D, NQ, 128).transpose(0, 1, 2)
                kslc = kT_full[b][:, off * 128 : (off + l) * 128]  # [D, l*128]
                pk_in[:, NQ : NQ + l, :] = kslc.reshape(D, l, 128)
                vslc = vh[b][off * 128 : (off + l) * 128]  # [l*128, D]
                # v packed partition-major: [p, i, d] = v[i*128+p, d]
                pk_in[:, NQ + t : NQ + t + l, :] = vslc.reshape(l, 128, D).transpose(1, 0, 2)
                keys = (off * 128 + kidx[: l * 128]).reshape(l, 128).T  # [128, l]
                if lb == 0:
                    sc[:, :l] = 0.0
                    bi[:, :l] = 0.0
                else:
                    mk = (keys < lb).astype(np.float32)
                    sc[:, :l] = mk * np.float32(SCALE)
                    bi[:, :l] = (1.0 - mk) * np.float32(NEG_BIAS)
            im[f"in{s}"] = np.ascontiguousarray(pk_in.reshape(128, -1))
            scbi_parts.append(np.concatenate([sc, bi], axis=1))
        im["scbi"] = np.ascontiguousarray(np.concatenate(scbi_parts, axis=1))
        in_maps.append(im)

    res = run_bass_kernel_spmd(nc, in_maps, list(range(NCORES)))
    global last_result
    last_result = res

    acc_o = np.zeros((B * 2, D, QCH), np.float32)
    acc_d = np.zeros((B * 2, QCH), np.float32)
    for c in range(NCORES):
        r = res.results[c]
        for s in range(len(caps)):
            chunk = assign[c][s]
            if chunk is None:
                continue
            u = chunk[0]
            acc_o[u] += r[f"oT{s}"].astype(np.float32)
            acc_d[u] += r[f"esum{s}"].astype(np.float32).sum(axis=0)

    out = np.empty((B, Q, D), np.float32)
    for u in range(B * 2):
        b, h = u // 2, u % 2
        out[b, h * QCH : (h + 1) * QCH] = (acc_o[u] / acc_d[u][None, :]).T
    return out
